# revision 62
# baseline (speedup 1.0000x reference)
"""Trainium2 Bass kernel for nn_Attention_47545287967487.

Causal multi-head attention (B=2, S=2048, D=1024, H=16, DH=64) with QK
RMS-norm, distributed over 8 NeuronCores via head tensor-parallelism:
each core owns 2 heads (a 128-column slice of Wq/Wk/Wv), computes its
two heads' attention output for ALL rows, then an AllToAll exchanges
head-slices for row-slices (1MB bf16 total vs the 16.8MB fp32
ReduceScatter this replaces); each core finishes with a full-Wo output
projection for its own 512-row block, written transposed ([D, 512]) so
the store is contiguous; the host transposes back.

Numerics: everything runs in bf16 (x, weights, attention internals,
output), fp32 in PSUM and for the RMS-norm statistics chain. Scores are
bounded (|q.k|/8 <= 8 after RMS-norm) so softmax skips the
max-subtraction pass.

Engine plan per core:
 - PE: x@W projections (bf16, K-tiled), QK^T with the two heads packed
   into array row-groups (tile_position), PV as [v|1]^T @ P so the
   softmax denominator is a free 65th output row, the local output
   projection (K=1024 over the 8 exchanged feature blocks), one merged
   q+k sum-of-squares matmul pair (zero-padded selectors accumulate
   both into one [4,512] PSUM tile), and 128x128 transposes of v.
 - ACT: exp (softmax), Square, and one merged rstd chain per chunk:
   rstd = exp(-0.5*ln(mean+eps)) over [4,512] (q and k head-groups
   together). All functions live in one pinned ACT table.
 - DVE: PSUM->SBUF casts, the q normalize (scalar_tensor_tensor fusing
   the 1/sqrt(DH) scale), the k normalize (fused into the k cast), the
   softmax-denominator reciprocal + normalize muls, mask muls.
 - GPSIMD: partition-broadcast of the denominator reciprocal, constant
   fills (broadcast of rstd rows stays on PE via K=4 selector matmuls --
   GPSIMD is ~3x slower per element and sits on the critical path there).

Scheduling: the emission order software-pipelines at three levels.
Within a chunk's attention, QK(j+1) and QK(j+2) sit between QK(j) and
PV(j) so PE never waits on the exp->mask chain, and diagonal tiles go
first so their 3-engine chains get the off-diagonal stretch as slack.
Across phases, each pipeline step runs: attention head (2 QK tile-pairs,
giving ACT exp work immediately), the next chunk's q/k projection
matmuls + rstd chain (Ln/Exp land early in the ACT queue, ahead of
~10us of softmax exps), then the attention remainder with the v
transposes, q/k normalizes and output-projection half-slabs woven into
its PE bubbles (the attention inner loop is ACT-throughput-bound, so PE
has ~300ns of idle per tile-pair that these fill). The AllToAll +
output projection of repetition r is woven through repetition r+1.

DMA instruction count is kept low (~56/iteration) because the DMA-issue
queues and the hardware DGE are instruction-rate-limited (~0.6-1.2us
per issue): x loads are 4 merged 2-k-tile DMAs per chunk.

kernel(**inputs) takes the FULL unsharded inputs and returns the FULL
[2, 2048, 1024] float32 output.
"""

import math
import numpy as np

import concourse.bacc as bacc
import concourse.mybir as mybir
from concourse import tile
from concourse.bass_utils import run_bass_kernel_spmd

import ml_dtypes

BF16 = ml_dtypes.bfloat16

# Problem shape (hardcoded per the harness contract).
B, S, D, DH = 2, 2048, 1024, 64
H = D // DH
N_CORES = 8
HEADS_PER_CORE = H // N_CORES          # 2
DC = HEADS_PER_CORE * DH               # 128 feature columns per core
EPS = 1e-6

SCHUNK = 512                            # s-chunk width
TT = 128                                # t-tile width
KT = D // 128                           # 8 contraction tiles
NCH = S // SCHUNK                       # 4 s-chunks per batch
ROWS = B * S                            # 4096
ROWS_PER_CORE = ROWS // N_CORES         # 512
NCHUNKS = B * NCH                       # 8 chunks, one per core after a2a

F32 = mybir.dt.float32
F32R = mybir.dt.float32r
BF = mybir.dt.bfloat16

# All ACT functions this kernel uses (Square, Ln, Exp, Copy) live in the
# 'natural_log_exp_and_others' table. The default table chooser picks the
# first table containing each function, which thrashes between the exp and
# ln tables (~1.3us per reload, dozens of reloads). Pin the chooser to the
# one table that covers everything by emptying the others (positions are
# preserved so act_func_set_id still indexes act_info.json correctly).
_PINNED_ACT_TABLE = "natural_log_exp_and_others"
_orig_get_act_tables = bacc.get_activation_tables


def _pinned_act_tables(arch):
    tables = _orig_get_act_tables(arch)
    return {
        name: (funcs if name == _PINNED_ACT_TABLE else set())
        for name, funcs in tables.items()
    }


bacc.get_activation_tables = _pinned_act_tables


def build_nc(collective=True, stage=3, repeat=1):
    nc = bacc.Bacc("TRN2", target_bir_lowering=False)

    xt_d = nc.dram_tensor("xt", [D, ROWS], BF, kind="ExternalInput")
    wq_d = nc.dram_tensor("wq", [D, DC], BF, kind="ExternalInput")
    wk_d = nc.dram_tensor("wk", [D, DC], BF, kind="ExternalInput")
    wv_d = nc.dram_tensor("wv", [D, DC], BF, kind="ExternalInput")
    wo_d = nc.dram_tensor("wo", [D, D], BF, kind="ExternalInput")
    mask_d = nc.dram_tensor("mask0", [TT, SCHUNK], BF, kind="ExternalInput")
    ident_d = nc.dram_tensor("ident", [128, 128], BF, kind="ExternalInput")
    # sel4 cols 0:4 select q head-groups into rows 0:2 (cols 2:4 zero);
    # cols 4:8 select k head-groups into rows 2:4 (cols 4:6 zero). The two
    # sumsq matmuls accumulate into one [4,512] PSUM tile.
    sel4_d = nc.dram_tensor("sel4", [128, 8], BF, kind="ExternalInput")
    # sel4t rows 0:4 / 4:8: K=4 selector that expands rstd's q / k rows
    # back to [DC, 512] via one PE matmul each.
    sel4t_d = nc.dram_tensor("sel4t", [8, 128], F32R, kind="ExternalInput")
    # Output is y^T for this core's 512-row block: [D, 512] bf16; the host
    # transposes back. Contiguous 512-col stores per out-feature slab.
    out_d = nc.dram_tensor("out", [D, ROWS_PER_CORE], BF, kind="ExternalOutput")

    from contextlib import ExitStack
    with tile.TileContext(nc) as tc:
        with ExitStack() as ctx:
            consts = ctx.enter_context(tc.tile_pool(name="consts", bufs=1))
            wpool = ctx.enter_context(tc.tile_pool(name="wpool", bufs=1))
            persist = ctx.enter_context(tc.tile_pool(name="persist", bufs=1))
            xcp = ctx.enter_context(tc.tile_pool(name="xc", bufs=4))
            sqp = ctx.enter_context(tc.tile_pool(name="sqp", bufs=4))
            stdp = ctx.enter_context(tc.tile_pool(name="stdp", bufs=4))
            bcp = ctx.enter_context(tc.tile_pool(name="bcp", bufs=6))
            vtp = ctx.enter_context(tc.tile_pool(name="vtp", bufs=3))
            vaugp = ctx.enter_context(tc.tile_pool(name="vaugp", bufs=12))
            pp = ctx.enter_context(tc.tile_pool(name="pp", bufs=16))
            zbp = ctx.enter_context(tc.tile_pool(name="zbp", bufs=6))
            rcp = ctx.enter_context(tc.tile_pool(name="rcp", bufs=6))
            attallp = ctx.enter_context(tc.tile_pool(name="attall", bufs=3))
            rcvp = ctx.enter_context(tc.tile_pool(name="rcvp", bufs=16))
            outsbp = ctx.enter_context(tc.tile_pool(name="outsb", bufs=4))
            ps_acc = ctx.enter_context(tc.tile_pool(name="ps_acc", bufs=2, space="PSUM"))
            ps_pt = ctx.enter_context(tc.tile_pool(name="ps_pt", bufs=3, space="PSUM"))
            ps_att = ctx.enter_context(tc.tile_pool(name="ps_att", bufs=2, space="PSUM"))
            # ss [4,512] and the bc broadcasts [128,512] share one rotating
            # bank: ss(ci) -> bcq(ci) -> bck(ci) -> ss(ci+1), each freed by
            # its single reader before the next needs the bank
            ps_ss = ctx.enter_context(tc.tile_pool(name="ps_ss", bufs=1, space="PSUM"))
            ps_bc = ps_ss
            dram = ctx.enter_context(tc.tile_pool(name="dram", bufs=4, space="DRAM"))

            # ---- weights first (gate the first projections), then consts.
            # One merged DMA per qkv weight: SBUF [128, KT*DC] where block k
            # holds DRAM rows [128k, 128k+128) so lhsT slices are [K=128, DC].
            w_sb = {}
            for wname, wd in (("q", wq_d), ("k", wk_d), ("v", wv_d)):
                t = wpool.tile([128, KT * DC], BF, name=f"w{wname}")
                nc.sync.dma_start(
                    t[:].rearrange("p (k c) -> p k c", k=KT),
                    wd[:].rearrange("(k p) c -> p k c", p=128))
                for k in range(KT):
                    w_sb[(wname, k)] = t[:, k * DC:(k + 1) * DC]

            sel4_sb = consts.tile([128, 8], BF, name="sel4_sb")
            nc.sync.dma_start(sel4_sb[:], sel4_d[:])
            sel4t_q = consts.tile([4, 128], F32R, name="sel4t_q")
            nc.sync.dma_start(sel4t_q[:], sel4t_d[0:4, :])
            sel4t_k = consts.tile([4, 128], F32R, name="sel4t_k")
            nc.sync.dma_start(sel4t_k[:], sel4t_d[4:8, :])
            ident_sb = consts.tile([128, 128], BF, name="ident_sb")
            nc.sync.dma_start(ident_sb[:], ident_d[:])
            mask_sb = consts.tile([TT, SCHUNK], BF, name="mask_sb")
            nc.sync.dma_start(mask_sb[:], mask_d[:])
            eps_sb = consts.tile([128, 1], F32, name="eps_sb")
            nc.vector.memset(eps_sb[:], EPS)
            # Full Wo, feature-block major: wo_sb[p, j*D + n*128 + c] =
            # Wo[j*128+p, n*128+c]; lhsT for (j, n) is a [128, 128] slice.
            # 8 DMAs (256KB each) so they spread across DMA engines.
            wo_sb = wpool.tile([128, KT * D], BF, name="wo_sb")
            for j in range(KT):
                nc.sync.dma_start(wo_sb[:, j * D:(j + 1) * D],
                                  wo_d[j * 128:(j + 1) * 128, :])

            # per-chunk q (normalized) / k (normalized) bf16, feature-major.
            qts = {}    # (b, i) -> [DC, SCHUNK] bf16
            kts = {}    # (b, i) -> [DC, SCHUNK] bf16
            vaug = {}   # (b, j) -> [128, 2*(DH+1)] bf16

            xcs = {}
            rep_box = [0]

            def prefetch_x(b, i):
                rep = rep_box[0]
                col0 = b * S + i * SCHUNK
                xc = xcp.tile([128, KT * SCHUNK], BF, name=f"x_{rep}_{b}_{i}",
                              tag="xc")
                # 4 DMAs of 2 k-tiles each: few issues (the DMA queues are
                # instruction-rate-limited) but still parallel across engines
                for k2 in range(KT // 2):
                    nc.sync.dma_start(
                        xc[:, 2 * k2 * SCHUNK:(2 * k2 + 2) * SCHUNK].rearrange(
                            "p (k c) -> p k c", k=2),
                        xt_d[256 * k2:256 * (k2 + 1),
                             col0:col0 + SCHUNK].rearrange(
                            "(k p) c -> p k c", p=128))
                xcs[(b, i)] = xc

            def proj_qk(b, i, xch):
                """QKV-side matmuls + the rstd statistics chain. Emitted
                BEFORE the previous chunk's attention so the Ln/Exp land
                early in the ACT queue (ahead of ~10us of softmax exps).
                Returns state for proj_norm."""
                rep = rep_box[0]
                psq = ps_acc.tile([DC, SCHUNK], F32, name=f"pq_{rep}_{b}_{i}",
                                  tag="acc")
                for k in range(KT):
                    nc.tensor.matmul(psq[:], w_sb[("q", k)][:], xch[k][:],
                                     start=(k == 0), stop=(k == KT - 1))
                sqq = sqp.tile([DC, SCHUNK], BF, name=f"sqq_{rep}_{b}_{i}",
                               tag="sq")
                nc.scalar.activation(sqq[:], psq[:],
                                     mybir.ActivationFunctionType.Square)

                psk = ps_acc.tile([DC, SCHUNK], F32, name=f"pk_{rep}_{b}_{i}",
                                  tag="acc")
                for k in range(KT):
                    nc.tensor.matmul(psk[:], w_sb[("k", k)][:], xch[k][:],
                                     start=(k == 0), stop=(k == KT - 1))
                # ss-q matmul sits AFTER the psk chain: ACT runs Square-q
                # during those 8 matmuls, so PE doesn't stall here
                ss = ps_ss.tile([4, SCHUNK], F32, name=f"ss_{rep}_{b}_{i}",
                                tag="ssbc")
                nc.tensor.matmul(ss[:], sel4_sb[:, 0:4], sqq[:],
                                 start=True, stop=False)
                sqk = sqp.tile([DC, SCHUNK], BF, name=f"sqk_{rep}_{b}_{i}",
                               tag="sq")
                nc.scalar.activation(sqk[:], psk[:],
                                     mybir.ActivationFunctionType.Square)
                nc.tensor.matmul(ss[:], sel4_sb[:, 4:8], sqk[:],
                                 start=False, stop=True)

                # rstd rows: [0:2] = q head-groups, [2:4] = k head-groups
                lm = stdp.tile([4, SCHUNK], F32, name=f"lm_{rep}_{b}_{i}",
                               tag="std")
                nc.scalar.activation(lm[:], ss[:],
                                     mybir.ActivationFunctionType.Ln,
                                     scale=1.0 / DH, bias=eps_sb[:4, :])
                rstd = stdp.tile([4, SCHUNK], F32R, name=f"rstd_{rep}_{b}_{i}",
                                 tag="std")
                nc.scalar.activation(rstd[:], lm[:],
                                     mybir.ActivationFunctionType.Exp,
                                     scale=-0.5)
                return psq, psk, rstd

            def norm_parts(b, i, psq, psk, rstd):
                """rstd broadcast + q/k normalize as two weave parts, placed
                mid-attention so PE reaches bcq well after ACT finished the
                rstd chain."""
                rep = rep_box[0]

                def nq():
                    bcq = ps_bc.tile([DC, SCHUNK], F32,
                                     name=f"bcq_{rep}_{b}_{i}", tag="ssbc")
                    nc.tensor.matmul(bcq[:], sel4t_q[:], rstd[:],
                                     start=True, stop=True)
                    bcqs = bcp.tile([DC, SCHUNK], F32,
                                    name=f"bcqs_{rep}_{b}_{i}", tag="bc")
                    nc.vector.tensor_copy(bcqs[:], bcq[:])
                    qtile = persist.tile([DC, SCHUNK], BF,
                                         name=f"qt_{rep}_{b}_{i}",
                                         tag="qtk", bufs=20)
                    qts[(b, i)] = qtile
                    # q also takes the 1/sqrt(DH) score scale here
                    nc.vector.scalar_tensor_tensor(
                        qtile[:], psq[:], 1.0 / math.sqrt(DH), bcqs[:],
                        mybir.AluOpType.mult, mybir.AluOpType.mult)

                def nk():
                    bck = ps_bc.tile([DC, SCHUNK], F32,
                                     name=f"bck_{rep}_{b}_{i}", tag="ssbc")
                    nc.tensor.matmul(bck[:], sel4t_k[:], rstd[:],
                                     start=True, stop=True)
                    bcks = bcp.tile([DC, SCHUNK], F32,
                                    name=f"bcks_{rep}_{b}_{i}", tag="bc")
                    nc.vector.tensor_copy(bcks[:], bck[:])
                    ktile = persist.tile([DC, SCHUNK], BF,
                                         name=f"kt_{rep}_{b}_{i}",
                                         tag="qtk", bufs=20)
                    kts[(b, i)] = ktile
                    nc.vector.tensor_mul(ktile[:], psk[:], bcks[:])

                return [nq, nk]

            def proj_v_mm(b, i, xch):
                rep = rep_box[0]
                psv = ps_acc.tile([DC, SCHUNK], F32, name=f"pv_{rep}_{b}_{i}",
                                  tag="acc")
                for k in range(KT):
                    nc.tensor.matmul(psv[:], w_sb[("v", k)][:], xch[k][:],
                                     start=(k == 0), stop=(k == KT - 1))
                vt = vtp.tile([DC, SCHUNK], BF, name=f"vt_{rep}_{b}_{i}",
                              tag="vt")
                nc.vector.tensor_copy(vt[:], psv[:])
                return vt

            def proj_v_tail(b, i, vt):
                # transposes emitted after proj_qk so the q/k matmuls hide
                # the psv->vt copy latency; all 4 t-tiles share one PSUM tile
                # and one [128, 4*2*65] vaug tile (one copy + one memset)
                rep = rep_box[0]
                nu = SCHUNK // TT
                tpt = ps_pt.tile([128, SCHUNK], BF, name=f"tp_{rep}_{b}_{i}",
                                 tag="pt")
                for u in range(nu):
                    nc.tensor.transpose(tpt[:, u * 128:(u + 1) * 128],
                                        vt[:, u * 128:(u + 1) * 128],
                                        ident_sb[:])
                va = vaugp.tile([128, nu * 2 * (DH + 1)], BF,
                                name=f"va_{rep}_{b}_{i}", tag="vaug")
                nc.vector.tensor_copy(
                    va[:].rearrange("p (u g e) -> p u g e", u=nu, g=2)[:, :, :, 0:DH],
                    tpt[:].rearrange("p (u g d) -> p u g d", u=nu, g=2))
                nc.gpsimd.memset(
                    va[:].rearrange("p (u g e) -> p u g e", u=nu, g=2)[:, :, :, DH:DH + 1],
                    1.0)
                for u in range(nu):
                    vaug[(b, i * nu + u)] = (va, u * 2 * (DH + 1))

            def proj_head(b, i):
                """Matmul-heavy first half of the projection step; returns
                the weave parts (v transposes, q/k normalizes) that fill PE
                bubbles inside the previous chunk's attention."""
                xc = xcs.pop((b, i))
                xch = [xc[:, k * SCHUNK:(k + 1) * SCHUNK] for k in range(KT)]
                vt = proj_v_mm(b, i, xch)
                st = proj_qk(b, i, xch)
                return [lambda: proj_v_tail(b, i, vt)], norm_parts(b, i, *st)

            def attn_parts(b, i, atx, lead=5):
                """Returns (head, rest): head emits the first `lead` QK
                tile-pairs (scheduled BEFORE the next chunk's projection
                matmuls so ACT gets exp work at step start), rest finishes
                the chunk."""
                rep = rep_box[0]
                att = [ps_att.tile([DH + 1, SCHUNK], F32,
                                   name=f"att_{rep}_{b}_{i}_{h}", tag="att")
                       for h in range(HEADS_PER_CORE)]
                n_t = 4 * i + 4

                def emit_qk(j):
                    off = max(0, TT * (j - 4 * i))
                    npx = SCHUNK - off
                    jc, ju = j // 4, j % 4
                    pts = []
                    for h in range(HEADS_PER_CORE):
                        pt = ps_pt.tile([128, SCHUNK], F32,
                                        name=f"ptile_{rep}_{b}_{i}_{j}_{h}",
                                        tag="pt")
                        nc.tensor.matmul(
                            pt[:, :npx],
                            kts[(b, jc)][h * DH:(h + 1) * DH,
                                         ju * TT:(ju + 1) * TT],
                            qts[(b, i)][h * DH:(h + 1) * DH, off:SCHUNK],
                            start=True, stop=True,
                            tile_position=(h * DH, 0),
                        )
                        pts.append(pt)
                    return pts

                def emit_pv(j, pts, first, last):
                    off = max(0, TT * (j - 4 * i))
                    npx = SCHUNK - off
                    va, vcol = vaug[(b, j)]
                    for h in range(HEADS_PER_CORE):
                        psb = pp.tile([128, SCHUNK], BF,
                                      name=f"p_{rep}_{b}_{i}_{j}_{h}", tag="p")
                        nc.scalar.activation(psb[:, :npx], pts[h][:, :npx],
                                             mybir.ActivationFunctionType.Exp)
                        if j >= 4 * i:
                            nc.vector.tensor_mul(psb[:, :npx], psb[:, :npx],
                                                 mask_sb[:, :npx])
                        nc.tensor.matmul(
                            att[h][:, off:SCHUNK],
                            va[:, vcol + h * (DH + 1):vcol + (h + 1) * (DH + 1)],
                            psb[:, :npx],
                            start=first, stop=last,
                        )

                # Diagonal tiles first: their 3-engine exp->mask->PV chains
                # get the whole off-diagonal stretch of slack. The j=4i tile
                # (off=0) goes first so its start=True covers the full att
                # range. QK(j') sits between QK(j) and PV(j) in the PE
                # stream so PE rarely waits on exp/mask.
                # One unmasked tile (j=0, off=0: full att coverage for its
                # start=True) leads so the head-popped PV below has a clean
                # exp->PV chain; diagonals follow, with the off-diagonal
                # stretch as slack for their exp->mask->PV chains.
                if i > 0:
                    order = [0] + list(range(4 * i, n_t)) + list(range(1, 4 * i))
                else:
                    order = list(range(4 * i, n_t))
                from collections import deque
                inflight = deque()
                nlead = min(lead, n_t)

                def head():
                    for n in range(nlead):
                        pts = emit_qk(order[n])
                        inflight.append((order[n], pts, n == 0, n == n_t - 1))

                def rest(weave=()):
                    wk = 0
                    for n in range(nlead, n_t):
                        pts = emit_qk(order[n])
                        inflight.append((order[n], pts, n == 0, n == n_t - 1))
                        if len(inflight) > 2:
                            emit_pv(*inflight.popleft())
                        if wk < len(weave):
                            weave[wk]()
                            wk += 1
                    while inflight:
                        emit_pv(*inflight.popleft())
                    while wk < len(weave):
                        weave[wk]()
                        wk += 1
                    # ---- normalize by softmax denominator; ship to atx ----
                    at_all = attallp.tile([DC, SCHUNK], BF,
                                          name=f"atall_{rep}_{b}_{i}", tag="attall")
                    for h in range(HEADS_PER_CORE):
                        rc = rcp.tile([1, SCHUNK], F32,
                                      name=f"rc_{rep}_{b}_{i}_{h}", tag="rc")
                        nc.vector.reciprocal(rc[:], att[h][DH:DH + 1, :])
                        zbs = zbp.tile([DH, SCHUNK], F32,
                                       name=f"zbs_{rep}_{b}_{i}_{h}", tag="zb")
                        nc.gpsimd.partition_broadcast(zbs[:], rc[:])
                        nc.vector.tensor_mul(at_all[h * DH:(h + 1) * DH, :],
                                             att[h][0:DH, :], zbs[:])

                    jd = b * NCH + i   # destination core / row-block index
                    nc.sync.dma_start(atx[jd * DC:(jd + 1) * DC, :], at_all[:])

                return head, rest

            def make_oparts(atx, rcv_d):
                """9 parts: [0] = a2a + rcv DMAs; [1..8] = out-feature slabs."""
                rep = rep_box[0]
                rcv_sb = [rcvp.tile([128, SCHUNK], BF,
                                    name=f"rcv_{rep}_{j}", tag="rcv")
                          for j in range(KT)]

                def exchange():
                    if collective:
                        nc.gpsimd.collective_compute(
                            "AllToAll",
                            mybir.AluOpType.bypass,
                            replica_groups=[list(range(N_CORES))],
                            ins=[atx[:]],
                            outs=[rcv_d[:]],
                        )
                        src = rcv_d
                    else:
                        src = atx  # TimelineSim variant: timing-equivalent
                    for j in range(KT):
                        nc.sync.dma_start(rcv_sb[j][:],
                                          src[j * 128:(j + 1) * 128, :])

                boxes = [dict() for _ in range(KT)]

                def slab_a(n):
                    op = ps_pt.tile([128, SCHUNK], F32,
                                    name=f"op_{rep}_{n}", tag="pt")
                    boxes[n]["op"] = op
                    for j in range(KT // 2):
                        nc.tensor.matmul(
                            op[:],
                            wo_sb[:, j * D + n * 128:j * D + (n + 1) * 128],
                            rcv_sb[j][:],
                            start=(j == 0), stop=False)

                def slab_b(n):
                    op = boxes[n]["op"]
                    for j in range(KT // 2, KT):
                        nc.tensor.matmul(
                            op[:],
                            wo_sb[:, j * D + n * 128:j * D + (n + 1) * 128],
                            rcv_sb[j][:],
                            start=False, stop=(j == KT - 1))
                    osb = outsbp.tile([128, SCHUNK], BF,
                                      name=f"osb_{rep}_{n}", tag="outsb")
                    nc.vector.tensor_copy(osb[:], op[:])
                    nc.sync.dma_start(out_d[n * 128:(n + 1) * 128, :], osb[:])

                parts = [exchange]
                for n in range(KT):
                    parts.append(lambda n=n: slab_a(n))
                    parts.append(lambda n=n: slab_b(n))
                return parts

            # Software pipeline. Per step: prefetch x for chunk ci+1,
            # attention for chunk ci-1, one output-projection part from the
            # PREVIOUS repetition (its AllToAll ran while this rep's first
            # chunks projected), then projections for chunk ci.
            chunks = [(b, i) for b in range(B) for i in range(NCH)]
            pending = []
            for rep_i in range(repeat):
                rep_box[0] = rep_i
                atx = dram.tile([NCHUNKS * DC, SCHUNK], BF,
                                name=f"atx_{rep_i}", tag="atx")
                rcv_d = dram.tile([NCHUNKS * DC, SCHUNK], BF,
                                  name=f"rcv_{rep_i}", tag="rcvd") \
                    if collective else None
                for ci in range(len(chunks) + 1):
                    if ci == 0:
                        if rep_i == 0:
                            prefetch_x(*chunks[0])
                            prefetch_x(*chunks[1])
                    elif ci + 1 < len(chunks):
                        prefetch_x(*chunks[ci + 1])
                    ah = ar = None
                    if ci >= 1 and stage >= 2:
                        ah, ar = attn_parts(*chunks[ci - 1], atx)
                        ah()
                    if ci < len(chunks):
                        early, norms = proj_head(*chunks[ci])
                    else:
                        early, norms = [], []
                    weave = list(early) + norms
                    # up to two oproj half-slabs per step as extra filler
                    for _ in range(2):
                        if pending:
                            weave.append(pending.pop(0))
                    if ar is not None:
                        ar(weave)
                    else:
                        for w in weave:
                            w()
                while pending:
                    pending.pop(0)()
                if rep_i + 1 < repeat:
                    # prefetch the next repetition's first two chunks so its
                    # projections never wait on DMA at the rep boundary
                    rep_box[0] = rep_i + 1
                    prefetch_x(*chunks[0])
                    prefetch_x(*chunks[1])
                    rep_box[0] = rep_i
                if stage >= 3:
                    pending = make_oparts(atx, rcv_d)
            for part in pending:
                part()
            if stage == 1:
                # flush qt/kt so the pipeline isn't dead code
                scratch = dram.tile([NCHUNKS * DC, SCHUNK], BF, name="scr")
                for n, ((b, i), t) in enumerate(list(qts.items())[:8]):
                    nc.sync.dma_start(scratch[n * DC:(n + 1) * DC, :], t[:])

    nc.compile()
    return nc


_NC_CACHE = {}


def _get_nc():
    if "nc" not in _NC_CACHE:
        _NC_CACHE["nc"] = build_nc()
    return _NC_CACHE["nc"]


def _host_inputs(x, Wq, Wk, Wv, Wo):
    xt = np.ascontiguousarray(
        np.asarray(x, dtype=np.float32).reshape(ROWS, D).T).astype(BF16)
    mask0 = (np.arange(TT)[:, None] <= np.arange(SCHUNK)[None, :]).astype(BF16)
    ident = np.eye(128, dtype=BF16)
    sel4 = np.zeros((128, 8), dtype=BF16)
    sel4[:DH, 0] = 1.0
    sel4[DH:2 * DH, 1] = 1.0
    sel4[:DH, 6] = 1.0
    sel4[DH:2 * DH, 7] = 1.0
    sel4t = np.zeros((8, 128), dtype=np.float32)
    sel4t[0, :DH] = 1.0
    sel4t[1, DH:2 * DH] = 1.0
    sel4t[4 + 2, :DH] = 1.0
    sel4t[4 + 3, DH:2 * DH] = 1.0
    wo = np.ascontiguousarray(np.asarray(Wo, dtype=np.float32)).astype(BF16)

    in_maps = []
    for c in range(N_CORES):
        cs = c * DC
        in_maps.append({
            "xt": xt,
            "wq": np.ascontiguousarray(Wq[:, cs:cs + DC]).astype(BF16),
            "wk": np.ascontiguousarray(Wk[:, cs:cs + DC]).astype(BF16),
            "wv": np.ascontiguousarray(Wv[:, cs:cs + DC]).astype(BF16),
            "wo": wo,
            "mask0": mask0,
            "ident": ident,
            "sel4": sel4,
            "sel4t": sel4t,
        })
    return in_maps


def kernel(x, Wq, Wk, Wv, Wo, mask):
    nc = _get_nc()
    in_maps = _host_inputs(x, np.asarray(Wq), np.asarray(Wk),
                           np.asarray(Wv), np.asarray(Wo))
    res = run_bass_kernel_spmd(nc, in_maps, list(range(N_CORES)))
    # Core c returns y^T [D, 512] for global rows [512c, 512c+512).
    full = np.concatenate(
        [np.asarray(res.results[c]["out"], dtype=np.float32).T
         for c in range(N_CORES)], axis=0)
    return full.reshape(B, S, D)


if __name__ == "__main__":
    nc = build_nc()
    print("kernel built and compiled OK")
